# revision 16
# baseline (speedup 1.0000x reference)
"""DistanceFromAnswerLoss on 8 Trainium2 NeuronCores — v3 (fp8/bf16 split).

out = 0.1 * sum_{b,c} mask[b,c] * exp(input[b,c])
  mask[b,c] = |c - t_b| / sqrt(sum_c (c - t_b)^2),  mask = 0 where t_b == 0

Host: rows sorted by t, 512/core, transposed (columns on partitions).
Per core a contiguous 16-block window covers every t; outside it
sign(c - t_b) is constant per 128-column block, so with m = window center

  sum_{c in agg} |c-t_b| e[c,b] = A~[b] + (m - t_b) * S~[b]
    A~ = sum +-(c-m) e,  S~ = sum +-e     (per-block 2-col matmuls)

Window blocks: weights are folded into exp's argument on the host
(|c-t| e^x = e^{x + ln|c-t|}); their sum rides the A~ PSUM row via a
[1, 0] stationary column.  Final: rows dotted with scale / scale*(m-t).

exp is split across two engines:
 - 32 aggregate blocks ship as fp8_e4m3 and run on ACT (dtype-blind
   1 elem/lane/cycle); an early dummy exp pulls the ~2.7us table load
   into the DMA spin-up dead time;
 - 16 aggregate + 16 window blocks ship as bf16 and run on the DVE as a
   Schraudolph bitcast exp (one 4x-mode tensor_scalar per chunk:
   e^x ~= bitcast_bf16(int16(x * 128/ln2 + (127*128 - CADJ)))).

The x stream is 12 uniform [128 x 4KB] transfers (fp8 chunk = 8 slots,
bf16 chunk = 4 slots; 6.29 MB total) on the single sync HWDGE ring —
mixed-size chunks provably unbalance the per-engine DMA queues and grow
a multi-us ragged tail.  ACT chunks are interleaved so the ACT chain
(longest engine, ~15us) never starves; window chunks stream last, and
the PSUM accumulation is split in two groups so all but the final 4
slots combine mid-stream.
"""

import sys
from contextlib import ExitStack

import numpy as np
import ml_dtypes

sys.path.insert(0, "/opt/trn_rl_repo")

import concourse.bass as bass
import concourse.tile as tile
from concourse import bacc, mybir
from concourse.bass_utils import run_bass_kernel_spmd

B = 4096
C = 8192
N_CORES = 8
ROWS = B // N_CORES          # 512 rows (free dim) per core
NQ = C // 128                # 64 column blocks of 128 (partition dim)
NS = 16                      # window blocks (log-baked weights, contiguous)
NAGG = NQ - NS               # 48 aggregate blocks
COEFF = 0.1

SCHR_SCALE = float(np.float32(128.0 / np.log(2.0)))
CADJ = 7.33                  # sawtooth centering (HW convert rounds)
SCHR_BIAS = float(np.float32(127.0 * 128.0 - CADJ))
LW_CLAMP = -50.0             # ln-weight clamp (e^{x-50} ~ 0)

# stream plan: A = 8 fp8 agg slots on ACT, D = 4 bf16 slots on DVE.
# Every chunk is a [128, 4KB] transfer.  A-chunks take agg blocks
# rest[0:32]; D-chunks take rest[32:48] then the 16 window blocks.
# STREAM_PLAN is DMA/exp order (ACT chunks early so the 14.8us ACT chain
# never starves); PE_ORDER is matmul order — the PE runs in program
# order, so A3's matmuls sit second-to-last (its exp lands last) to keep
# mid-stream D matmuls from stalling behind the ACT chain.
STREAM_PLAN = ["A0", "A1", "D0", "A2", "D1", "D2", "A3", "D3",
               "D4", "D5", "D6", "D7"]
PE_ORDER = ["A0", "A1", "D0", "A2", "D1", "D2", "D3", "D4",
            "D5", "D6", "A3", "D7"]
NA = 4                               # fp8 chunks (8 slots each)
ND = 8                               # bf16 chunks (4 slots each)
QCUT = NQ - 4                        # last PE chunk (D7) -> PSUM group B
N_PRIME = 8                          # dummy matmuls to pre-ramp PE p-state

F32 = mybir.dt.float32
BF16 = mybir.dt.bfloat16
FP8 = mybir.dt.float8e4
I16 = mybir.dt.int16
Op = mybir.AluOpType
Af = mybir.ActivationFunctionType


def _build() -> bass.Bass:
    nc = bacc.Bacc("TRN2", target_bir_lowering=False, debug=False)
    x8 = nc.declare_dram_parameter("x8", [128, NA * 8 * ROWS], FP8, isOutput=False)
    xb = nc.declare_dram_parameter("xb", [128, ND * 4 * ROWS], BF16, isOutput=False)
    wv = nc.declare_dram_parameter("wv", [128, 2 * NQ], BF16, isOutput=False)
    scs = nc.declare_dram_parameter("scs", [2, ROWS], F32, isOutput=False)
    out = nc.declare_dram_parameter("out", [2, 2], F32, isOutput=True)

    with tile.TileContext(nc) as tc, ExitStack() as ctx:
        const_pool = ctx.enter_context(tc.tile_pool(name="const", bufs=1))
        xpool = ctx.enter_context(tc.tile_pool(name="x", bufs=1))
        epool = ctx.enter_context(tc.tile_pool(name="e", bufs=1))
        spool = ctx.enter_context(tc.tile_pool(name="s", bufs=1))
        psum_pool = ctx.enter_context(tc.tile_pool(name="ps", bufs=1, space="PSUM"))

        # ACT table load happens during the DMA spin-up dead time
        warm = const_pool.tile([128, 1], BF16)
        nc.vector.memset(warm[:], 0.0)
        warme = const_pool.tile([128, 1], BF16)
        nc.scalar.activation(warme[:], warm[:], Af.Exp)

        # constants on the scalar HWDGE ring
        wvt = const_pool.tile([128, 2 * NQ], BF16)
        nc.scalar.dma_start(out=wvt[:], in_=wv[:, :])
        scst = const_pool.tile([2, ROWS], F32)
        nc.scalar.dma_start(out=scst[:], in_=scs[:, :])

        # x stream: uniform 4KB-per-partition chunks on the sync ring,
        # each followed by its exp (ACT for fp8 chunks, DVE Schraudolph
        # for bf16 chunks) in stream order
        xt, et = {}, {}
        a0 = d0 = 0
        for cid in STREAM_PLAN:
            if cid[0] == "A":
                t = xpool.tile([128, 8 * ROWS], FP8, name=f"x{cid}")
                nc.sync.dma_start(
                    out=t[:], in_=x8[:, a0 * ROWS:(a0 + 8) * ROWS]
                )
                a0 += 8
                e = epool.tile([128, 8 * ROWS], BF16, name=f"e{cid}")
                nc.scalar.activation(e[:], t[:], Af.Exp)
                eb = e[:]
            else:
                t = xpool.tile([128, 4 * ROWS], BF16, name=f"x{cid}")
                nc.sync.dma_start(
                    out=t[:], in_=xb[:, d0 * ROWS:(d0 + 4) * ROWS]
                )
                d0 += 4
                e = epool.tile([128, 4 * ROWS], I16, name=f"e{cid}")
                nc.vector.tensor_scalar(
                    e[:], t[:], SCHR_SCALE, SCHR_BIAS,
                    op0=Op.mult, op1=Op.add,
                )
                eb = e[:].bitcast(BF16)
            xt[cid], et[cid] = t, eb

        aspsA = psum_pool.tile([2, ROWS], F32, tag="pasA")
        aspsB = psum_pool.tile([2, ROWS], F32, tag="pasB")
        res = spool.tile([2, 2], F32)

        # PE p-state priming: dummy matmuls on the first chunk's raw bytes
        # (results discarded) so the real matmul stream starts at 2.4 GHz
        prps = psum_pool.tile([1, ROWS], F32, tag="prime")
        pm = xt[STREAM_PLAN[0]][:].bitcast(BF16)
        for _ in range(N_PRIME):
            nc.tensor.matmul(
                prps[:], wvt[:, 0:1], pm[:, 0:ROWS], start=True, stop=True,
                skip_group_check=True,
            )

        q = 0
        for cid in PE_ORDER:
            nsl = 8 if cid[0] == "A" else 4
            eb = et[cid]
            for k in range(nsl):
                ps = aspsA if q < QCUT else aspsB
                nc.tensor.matmul(
                    ps[:], wvt[:, 2 * q:2 * q + 2],
                    eb[:, k * ROWS:(k + 1) * ROWS],
                    start=(q == 0 or q == QCUT),
                    stop=(q == QCUT - 1 or q == NQ - 1),
                )
                if q == QCUT - 1:
                    # group A combine runs mid-stream, hidden under the tail
                    jA = spool.tile([2, ROWS], F32, name="jA")
                    nc.vector.scalar_tensor_tensor(
                        jA[:], aspsA[:], 0.0, scst[:],
                        op0=Op.add, op1=Op.mult, accum_out=res[:, 0:1],
                    )
                q += 1

        jB = spool.tile([2, ROWS], F32, name="jB")
        nc.vector.scalar_tensor_tensor(
            jB[:], aspsB[:], 0.0, scst[:], op0=Op.add, op1=Op.mult,
            accum_out=res[:, 1:2],
        )
        nc.sync.dma_start(out=out[:, :], in_=res[:])

    nc.finalize()
    return nc


_NC = None


def _get_nc() -> bass.Bass:
    global _NC
    if _NC is None:
        _NC = _build()
    return _NC


def _plan(target: np.ndarray):
    """Sort rows by target; per core pick a contiguous 16-block window
    covering all its targets and the agg/window block split."""
    t = np.asarray(target).astype(np.int64).reshape(B)
    order = np.argsort(t, kind="stable")
    plans = []
    for k in range(N_CORES):
        rows = order[k * ROWS:(k + 1) * ROWS]
        tc = t[rows]
        blo, bhi = int(tc.min()) >> 7, int(tc.max()) >> 7
        span = bhi - blo + 1
        assert span <= NS, f"target spread too wide for window: {span} blocks"
        wlo = min(max(blo - (NS - span) // 2, 0), NQ - NS)
        assert wlo <= blo and bhi < wlo + NS
        win = np.arange(wlo, wlo + NS)
        rest = np.array([q for q in range(NQ) if q < wlo or q >= wlo + NS])
        plans.append((rows, tc, win, rest))
    return plans


def _stream_slots(win, rest):
    """Block id per PE slot (PE_ORDER), and per-buffer block lists.

    A-chunk Ai covers rest[8i:8i+8] (fp8 buffer, stream order); D-chunk
    Di covers rest[32+4i:32+4i+4] for i<4 and win[4(i-4):...] for i>=4.
    """
    a_blocks = list(rest[:NA * 8])
    d_blocks = list(rest[NA * 8:]) + list(win)
    chunk_blocks = {}
    for i in range(NA):
        chunk_blocks[f"A{i}"] = a_blocks[8 * i:8 * i + 8]
    for i in range(ND):
        chunk_blocks[f"D{i}"] = d_blocks[4 * i:4 * i + 4]
    slots = []
    for cid in PE_ORDER:
        slots += chunk_blocks[cid]
    return np.array(slots), np.array(a_blocks), np.array(d_blocks)


def make_in_maps(input: np.ndarray, target: np.ndarray) -> list[dict]:
    xf = np.asarray(input, dtype=np.float32)
    plans = _plan(target)
    s1 = (C - 1) * C // 2
    s2 = (C - 1) * C * (2 * C - 1) // 6
    in_maps = []
    p128 = np.arange(128, dtype=np.float64)
    for rows, tc, win, rest in plans:
        slots, a_blocks, d_blocks = _stream_slots(win, rest)
        m = float(win[0] * 128 + (NS * 128) / 2.0)
        xr = xf[rows].reshape(ROWS, NQ, 128)

        # fp8 payload: 32 agg blocks in A-chunk order
        xa = xr[:, a_blocks, :]                      # [ROWS, 32, 128]
        x8 = np.ascontiguousarray(xa.transpose(2, 1, 0)).reshape(
            128, NA * 8 * ROWS
        ).astype(ml_dtypes.float8_e4m3)

        # bf16 payload: 16 agg blocks then 16 window blocks (D-chunk order)
        xd = xr[:, d_blocks, :].transpose(2, 1, 0).astype(np.float64)
        cw = win[None, :] * 128 + p128[:, None]       # [128, NS]
        dist = np.abs(cw[:, :, None] - tc[None, None, :].astype(np.float64))
        lw = np.log(np.maximum(dist, 1e-30))
        np.maximum(lw, LW_CLAMP, out=lw)
        xd[:, ND * 4 - NS:, :] += lw
        xbp = np.ascontiguousarray(xd).reshape(
            128, ND * 4 * ROWS
        ).astype(ml_dtypes.bfloat16)

        # stationary columns in stream-slot order
        is_win = np.isin(slots, win)
        sgn = np.where(slots * 128 > win[-1] * 128, 1.0, -1.0)
        cs = slots[None, :] * 128 + p128[:, None] - m   # [128, 64]
        wvc = np.zeros((128, 2 * NQ), dtype=np.float32)
        wvc[:, 0::2] = np.where(is_win[None, :], 1.0, cs * sgn[None, :])
        wvc[:, 1::2] = np.where(is_win[None, :], 0.0, sgn[None, :])

        norm = np.sqrt(C * tc.astype(np.float64) ** 2 - 2.0 * tc * s1 + s2)
        sc64 = COEFF / np.maximum(norm, 1e-12) * (tc != 0)
        scs = np.stack([sc64, sc64 * (m - tc.astype(np.float64))])
        in_maps.append({
            "x8": x8,
            "xb": xbp,
            "wv": wvc.astype(ml_dtypes.bfloat16),
            "scs": np.ascontiguousarray(scs.astype(np.float32)),
        })
    return in_maps


def run(input: np.ndarray, target: np.ndarray, trace: bool = False, tmpdir=None):
    nc = _get_nc()
    in_maps = make_in_maps(input, target)
    res = run_bass_kernel_spmd(
        nc, in_maps, list(range(N_CORES)), trace=trace, tmpdir=tmpdir
    )
    total = np.float32(0.0)
    for r in res.results:
        total += np.float32(r["out"].reshape(-1).sum())
    return np.asarray(total, dtype=np.float32), res


def kernel(input: np.ndarray, target: np.ndarray) -> np.ndarray:
    out, _ = run(input, target)
    return out
